# revision 1
# baseline (speedup 1.0000x reference)
"""Trainium2 Bass kernel for nn_DiscriminativeLoss.

Data-parallel over the batch axis: each of the 8 NeuronCores gets one sample
(input[b] of shape (32, 65536) plus target[b, 0] of shape (65536,)) and
computes the per-sample sufficient statistics on-chip:

  cnt0      = sum_n t0[n]
  s0[f]     = sum_n x[f,n] * t0[n]          (masked feature sums, cluster 0)
  rs[f]     = sum_n x[f,n]                  (total feature sums; s1 = rs - s0)
  m0,m1     = cluster means (safe-count divide, on-chip)
  v0        = sum_n max(||x_n - m0|| - dv, 0)^2 * t0[n]
  v1        = sum_n max(||x_n - m1|| - dv, 0)^2 * (1 - t0[n])

The host combines the 8 small per-core result vectors into the scalar loss
(the tiny all-reduce-mean step of the batch-parallel sharding).

On-chip layout (per core), n = 16384*jj + u = 512*p + q:
  X, Xsq [128, 16384] : partition (32*jj+f), free u
  T0n    [128, 512]   : partition p, free q
  T0cc   [128, 5632]  : partition (32*z+jj), free (512*g+q), chunk i = 11*z+g
Phase 1: per 512-col chunk, PE replicates t0 across the 32 f-partitions of
each quadrant (K=4 matmul from T0cc) and DVE tensor_tensor_reduce
accumulates the masked sums; row sums via ACT/DVE accumulate; squares into
Xsq split across ACT/DVE/GPSIMD. After the means are formed on-chip, phase 2
computes dist_c^2 - ||m_c||^2 directly in PSUM with two accumulating fp32r
matmuls per chunk (block -2*m_c weights over X, then block-ones over Xsq),
packing 3 chunks per PSUM bank (bases 0/32/64) so one [96,512] engine copy
evacuates 3 chunks; a SBUF->SBUF DMA re-lays the rows into the n-major
D01 [128, 1024] tile where the sqrt/hinge chain and masked reductions run.
"""

import numpy as np
from contextlib import ExitStack

BS, NF, MAXC, NLOC = 8, 32, 4, 65536
DELTA_VAR, DELTA_DIST = 0.5, 1.5
ALPHA, BETA, GAMMA = 1.0, 1.0, 1e-4

NCORES = 8
CH = 32          # 512-column chunks per core
CW = 512         # chunk width
U = NLOC // 4    # 16384 columns per quadrant
NG = 11          # chunks per z-group (CH = 3*11 - 1)

_CACHE = {}

# engine splits (tuned against trace): chunk index -> engine
RS_ENGINE = ["act"] * 26 + ["dve"] * 6
SQ_ENGINE = (["gps", "gps", "dve"] * 6 + ["gps", "act"] * 7)[:32]
EVAC_ENGINE = ["act", "dve"] * 6


def _zg(i):
    return i // NG, i % NG


def _host_constants():
    sel33 = np.zeros((128, 33), dtype=np.float32)
    for p in range(128):
        sel33[p, p % 32] = 1.0
    sel33[:, 32] = 1.0
    ones33 = np.ones((128, 33), dtype=np.float32)
    # cw1p: block-ones, col 2*jj+c (c=0,1) selects quadrant jj; cols 8..32 zero
    w1p = np.zeros((128, 32), dtype=np.float32)
    for jj in range(4):
        for c in range(2):
            w1p[32 * jj:32 * jj + 32, 2 * jj + c] = 1.0
    # replicated at partition bases 0/32/64 so lhsT base matches rhs base
    lhsT4 = np.zeros((128, 128), dtype=np.float32)
    for z in range(3):
        for jj in range(4):
            lhsT4[32 * z + jj, 32 * jj:32 * jj + 32] = 1.0
    import ml_dtypes
    cst = np.concatenate([sel33, ones33], axis=1)  # [128, 66] fp32
    cstb = np.concatenate([w1p, lhsT4], axis=1).astype(ml_dtypes.bfloat16)  # [128, 160]
    return {"cst": cst, "cstb": cstb}


def _emit(ctx, tc, x_d, t0_d, t0b_d, cst_d, cstb_d, res_d):
    import concourse.mybir as mybir

    nc = tc.nc
    f32 = mybir.dt.float32
    bf16 = mybir.dt.bfloat16
    Alu = mybir.AluOpType
    Act = mybir.ActivationFunctionType
    AxX = mybir.AxisListType.X

    persist = ctx.enter_context(tc.tile_pool(name="persist", bufs=1))
    scratch = ctx.enter_context(tc.tile_pool(name="scratch", bufs=1))
    stage_p = ctx.enter_context(tc.tile_pool(name="stage", bufs=2))
    p_t0rep = ctx.enter_context(tc.tile_pool(name="p_t0rep", bufs=2, space="PSUM"))
    p_dist = ctx.enter_context(tc.tile_pool(name="p_dist", bufs=2, space="PSUM"))
    p_fin = ctx.enter_context(tc.tile_pool(name="p_fin", bufs=2, space="PSUM"))

    def ptile(shape, tag, dtype=f32):
        return persist.tile(shape, dtype, tag=tag, name=tag)

    # ---- persistent tiles ----
    X = ptile([128, U], "X")
    Xb = ptile([128, U], "Xb", dtype=bf16)
    Xsqb = ptile([128, U], "Xsqb", dtype=bf16)
    T0ccb = ptile([128, NG * CW], "T0ccb", dtype=bf16)
    # masks in the D01 partition layout: P = 4*i + jj, value t0[16384*jj+512*i+q]
    TT01 = ptile([128, 2 * CW], "TT01")    # [T0n | T1n]
    T0n = TT01[:, 0:CW]
    T1n = TT01[:, CW:2 * CW]
    D01 = ptile([128, 2 * CW], "D01")
    CST = ptile([128, 66], "CST")          # [csel | cone] fp32
    csel = CST[:, 0:33]
    cone = CST[:, 33:66]
    CSTB = ptile([128, 160], "CSTB", dtype=bf16)   # [cw1p | clhst4] bf16
    cw1p = CSTB[:, 0:32]
    clhst4 = CSTB[:, 32:160]
    W2B = ptile([128, 32], "W2B", dtype=bf16)
    mnegb = ptile([32, 2], "mnegb", dtype=bf16)
    s0pc = ptile([128, CH], "s0pc")
    rspc = ptile([128, CH], "rspc")
    MISC = ptile([128, 64], "MISC")
    stats3 = MISC[:, 32:35]
    vstats = MISC[:, 35:37]
    stat_f = MISC[0:33, 37:40]
    stat_t = MISC[0:33, 40:43]
    cnts = MISC[0:32, 43:47]               # cols: cnt0s, cnt1s, rc0, rc1
    mraw = MISC[0:32, 47:49]
    mneg = MISC[0:32, 49:51]
    csb = MISC[0:1, 51:53]
    vout = MISC[0:1, 53:55]
    bias01 = MISC[:, 55:57]
    biasdv = MISC[:, 57:58]
    s1col = MISC[0:32, 58:59]

    # ---- loads ----
    x_ap = x_d.ap()
    t0_ap = t0_d.ap()
    nc.gpsimd.memset(biasdv, -DELTA_VAR)
    nc.sync.dma_start(T0n, t0_ap.rearrange("(jj i q) -> i jj q", jj=4, i=32))
    # T0ccb[32*z+jj, 512*g+q] = t0[16384*jj + 512*(11*z+g) + q]  (bf16, exact)
    t0b_ap = t0b_d.ap()
    for z in range(3):
        span = NG * CW if z < 2 else (CH - 2 * NG) * CW
        for jj in range(4):
            nc.scalar.dma_start(
                T0ccb[32 * z + jj:32 * z + jj + 1, 0:span],
                t0b_ap[U * jj + NG * CW * z: U * jj + NG * CW * z + span],
            )
    nc.sync.dma_start(CST[:], cst_d.ap())
    nc.sync.dma_start(CSTB[:], cstb_d.ap())
    # column-major windows so phase-1 chunks can start as soon as their
    # window (all 4 quadrants) has landed; spread across the 3 DGE queues
    dma_engines = [nc.sync, nc.scalar]
    WW = 2048
    for w in range(U // WW):
        for jj in range(4):
            dma_engines[jj % 2].dma_start(
                X[32 * jj:32 * jj + 32, w * WW:(w + 1) * WW],
                x_ap[:, jj * U + w * WW: jj * U + (w + 1) * WW],
            )

    # ---- phase 1 ----
    for i in range(CH):
        z, g = _zg(i)
        xs = X[:, i * CW:(i + 1) * CW]
        t0rep = p_t0rep.tile([128, CW], f32, tag="t0rep")
        nc.tensor.matmul(
            t0rep[:], clhst4[32 * z:32 * z + 4, :],
            T0ccb[32 * z:32 * z + 4, g * CW:(g + 1) * CW],
            start=True, stop=True,
        )
        # custom DVE ops (ttr) cannot read PSUM on HW: plain TT then reduce
        scr = scratch.tile([128, CW], f32, tag="scr_ttr", name="scr")
        nc.vector.tensor_tensor(out=scr[:], in0=xs, in1=t0rep[:], op=Alu.mult)
        nc.vector.reduce_sum(s0pc[:, i:i + 1], scr[:], axis=AxX)
    # ---- rs cast + squares in 2048-wide chunks (4x fewer ops) ----
    BW = 4 * CW
    for w in range(U // BW):
        xs = X[:, w * BW:(w + 1) * BW]
        # rs + bf16 cast fused: ACT copy X -> Xb with accumulate
        nc.scalar.activation(
            out=Xb[:, w * BW:(w + 1) * BW], in_=xs, func=Act.Copy,
            accum_out=rspc[:, w:w + 1])
        xq = Xsqb[:, w * BW:(w + 1) * BW]
        eng = ["gps", "dve", "gps", "act", "gps", "dve", "gps", "act"][w]
        if eng == "act":
            nc.scalar.activation(out=xq, in_=xs, func=Act.Square)
        elif eng == "dve":
            nc.vector.tensor_tensor(out=xq, in0=xs, in1=xs, op=Alu.mult)
        else:
            nc.gpsimd.tensor_tensor(out=xq, in0=xs, in1=xs, op=Alu.mult)

    # ---- cross-partition finish #1 (per-f sums + totals) ----
    nc.vector.reduce_sum(stats3[:, 0:1], s0pc[:], axis=AxX)
    nc.vector.reduce_sum(stats3[:, 1:2], rspc[:, 0:8], axis=AxX)
    nc.vector.reduce_sum(stats3[:, 2:3], T0n, axis=AxX)
    F1f = p_fin.tile([33, 3], f32, tag="fin")
    nc.tensor.matmul(F1f[:], csel, stats3, start=True, stop=True)
    F1t = p_fin.tile([33, 3], f32, tag="fin")
    nc.tensor.matmul(F1t[:], cone, stats3, start=True, stop=True)
    nc.scalar.copy(stat_f[:], F1f[:])
    nc.scalar.copy(stat_t[:], F1t[:])

    # ---- means (f on partitions 0..31; totals available on every row) ----
    cnt0col = stat_t[0:32, 2:3]
    nc.vector.tensor_scalar(
        out=cnts[:, 0:1], in0=cnt0col, scalar1=1.0, scalar2=None, op0=Alu.max)
    nc.vector.tensor_scalar(
        out=cnts[:, 1:2], in0=cnt0col, scalar1=-1.0, scalar2=float(NLOC),
        op0=Alu.mult, op1=Alu.add)
    nc.vector.tensor_scalar(
        out=cnts[:, 1:2], in0=cnts[:, 1:2], scalar1=1.0, scalar2=None, op0=Alu.max)
    nc.vector.reciprocal(cnts[:, 2:3], cnts[:, 0:1])
    nc.vector.reciprocal(cnts[:, 3:4], cnts[:, 1:2])
    nc.vector.tensor_tensor(
        out=mraw[:, 0:1], in0=stat_f[0:32, 0:1], in1=cnts[:, 2:3], op=Alu.mult)
    nc.vector.tensor_tensor(
        out=s1col, in0=stat_f[0:32, 1:2], in1=stat_f[0:32, 0:1], op=Alu.subtract)
    nc.vector.tensor_tensor(
        out=mraw[:, 1:2], in0=s1col, in1=cnts[:, 3:4], op=Alu.mult)
    nc.vector.tensor_scalar(
        out=mneg, in0=mraw, scalar1=-2.0, scalar2=None, op0=Alu.mult)

    # W2B: block-diagonal -2*m_c weights (bf16) at cols 2*jj+c (others zero)
    nc.vector.tensor_copy(mnegb, mneg)
    nc.gpsimd.memset(W2B[:], 0.0)
    for jj in range(4):
        nc.sync.dma_start(W2B[32 * jj:32 * jj + 32, 2 * jj:2 * jj + 2], mnegb)

    # ||m_c||^2 -> bias01 on all partitions
    mm0 = p_fin.tile([1, 2], f32, tag="fin")
    nc.tensor.matmul(mm0[:], mraw[:, 0:1], mraw, start=True, stop=True)
    mm1 = p_fin.tile([1, 2], f32, tag="fin")
    nc.tensor.matmul(mm1[:], mraw[:, 1:2], mraw, start=True, stop=True)
    nc.scalar.copy(csb[0:1, 0:1], mm0[0:1, 0:1])
    nc.scalar.copy(csb[0:1, 1:2], mm1[0:1, 1:2])
    nc.gpsimd.partition_broadcast(bias01[:], csb, channels=128)

    res_ap = res_d.ap()
    nc.sync.dma_start(res_ap[0:64].rearrange("(c f) -> f c", c=2), mraw)

    # ---- phase 2: dist_c^2 - ||m_c||^2 into PSUM, 3 chunks per bank ----
    for g in range(NG):
        nz = 3 if g < CH - 2 * NG else 2
        pd = p_dist.tile([128, CW], f32, tag="dist")
        for z in range(nz):
            i = NG * z + g
            nc.tensor.matmul(
                pd[32 * z:32 * z + 32, :], W2B[:],
                Xb[:, i * CW:(i + 1) * CW], start=True, stop=False)
            nc.tensor.matmul(
                pd[32 * z:32 * z + 32, :], cw1p,
                Xsqb[:, i * CW:(i + 1) * CW], start=False, stop=True)
        stg = stage_p.tile([128, CW], f32, tag="stg")
        if EVAC_ENGINE[g % len(EVAC_ENGINE)] == "act":
            nc.scalar.copy(stg[0:32 * nz, :], pd[0:32 * nz, :])
        else:
            nc.vector.tensor_copy(stg[0:32 * nz, :], pd[0:32 * nz, :])
        # widen into D01 with partition P = 4*i + jj (i = 11*z + g):
        # D01[4*i + jj, 512*c + q] = stg[32*z + 2*jj + c, q]; contiguous dst
        for z in range(nz):
            i = NG * z + g
            dst = D01[4 * i:4 * i + 4, :].rearrange("p (c q) -> p c q", c=2)
            (nc.sync if (g + z) % 2 == 0 else nc.scalar).dma_start(
                dst, stg[32 * z:32 * z + 8, :])

    # ---- hinge chain on D01 (in place) ----
    nc.vector.tensor_scalar(
        out=T1n, in0=T0n, scalar1=-1.0, scalar2=1.0,
        op0=Alu.mult, op1=Alu.add)
    for c in range(2):
        half = D01[:, c * CW:(c + 1) * CW]
        nc.vector.tensor_scalar(
            out=half, in0=half, scalar1=bias01[:, c:c + 1], scalar2=0.0,
            op0=Alu.add, op1=Alu.max)
    nc.scalar.activation(out=D01, in_=D01, func=Act.Sqrt)
    nc.scalar.activation(out=D01, in_=D01, func=Act.Relu, bias=biasdv[:, 0:1])
    nc.scalar.activation(out=D01, in_=D01, func=Act.Square)
    for c in range(2):
        scr = scratch.tile([128, CW], f32, tag="scr_ttr", name="scr")
        nc.vector.tensor_tensor(
            out=scr[:], in0=D01[:, c * CW:(c + 1) * CW],
            in1=(T0n if c == 0 else T1n), op=Alu.mult)
        nc.vector.reduce_sum(vstats[:, c:c + 1], scr[:], axis=AxX)

    # ---- final sums + outputs ----
    F2 = p_fin.tile([1, 2], f32, tag="fin")
    nc.tensor.matmul(F2[:], cone[:, 0:1], vstats, start=True, stop=True)
    nc.scalar.copy(vout, F2[0:1, 0:2])
    nc.sync.dma_start(res_ap[64:67], stat_t[0:1, 0:3])
    nc.sync.dma_start(res_ap[67:69], vout[0:1, 0:2])
    nc.sync.dma_start(res_ap[69:71], csb[0:1, 0:2])


def _build():
    import concourse.bacc as bacc
    import concourse.tile as tile
    import concourse.mybir as mybir

    f32 = mybir.dt.float32
    nc = bacc.Bacc("TRN2", target_bir_lowering=False, debug=False)
    x_d = nc.dram_tensor("x", [NF, NLOC], f32, kind="ExternalInput")
    t0_d = nc.dram_tensor("t0", [NLOC], f32, kind="ExternalInput")
    t0b_d = nc.dram_tensor("t0b", [NLOC], mybir.dt.bfloat16, kind="ExternalInput")
    cst_d = nc.dram_tensor("cst", [128, 66], f32, kind="ExternalInput")
    cstb_d = nc.dram_tensor("cstb", [128, 160], mybir.dt.bfloat16,
                            kind="ExternalInput")
    res_d = nc.dram_tensor("res", [128], f32, kind="ExternalOutput")
    with tile.TileContext(nc) as tc:
        with ExitStack() as ctx:
            _emit(ctx, tc, x_d, t0_d, t0b_d, cst_d, cstb_d, res_d)
    nc.compile()
    return nc


def get_nc():
    if "nc" not in _CACHE:
        _CACHE["nc"] = _build()
    return _CACHE["nc"]


def make_in_maps(input, target):
    consts = _host_constants()
    in_maps = []
    for b in range(input.shape[0]):
        import ml_dtypes
        t0 = np.ascontiguousarray(target[b, 0], dtype=np.float32)
        m = {
            "x": np.ascontiguousarray(input[b], dtype=np.float32),
            "t0": t0,
            "t0b": t0.astype(ml_dtypes.bfloat16),
        }
        m.update(consts)
        in_maps.append(m)
    return in_maps


def combine_host(results, n_clusters):
    """results: list of 8 dicts with 'res' vectors. Returns scalar loss."""
    total = 0.0
    for b in range(BS):
        r = np.asarray(results[b]["res"], dtype=np.float64)
        m0, m1 = r[0:32], r[32:64]
        cnt0 = r[66]
        v0, v1 = r[67], r[68]
        ncb = float(n_clusters[b])
        counts = np.array([cnt0, NLOC - cnt0])
        active = counts > 0
        safe = np.where(active, counts, 1.0)
        c_var = float(np.where(active, np.array([v0, v1]) / safe, 0.0).sum())
        l_var = c_var / ncb
        dn = float(np.sqrt(((m0 - m1) ** 2).sum()))
        c_dist = 2.0 * max(2.0 * DELTA_DIST - dn, 0.0) ** 2
        l_dist = c_dist / (2.0 * ncb * (ncb - 1.0))
        l_reg = 0.5 * (np.sqrt((m0 ** 2).sum()) + np.sqrt((m1 ** 2).sum()))
        total += ALPHA * l_var + BETA * l_dist + GAMMA * l_reg
    return np.float32(total / BS)


def kernel(input, target, n_clusters):
    from concourse import bass_utils

    nc = get_nc()
    in_maps = make_in_maps(np.asarray(input), np.asarray(target))
    br = bass_utils.run_bass_kernel_spmd(nc, in_maps, core_ids=list(range(NCORES)))
    loss = combine_host(br.results, np.asarray(n_clusters))
    return np.array(loss, dtype=np.float32)



# revision 15
# speedup vs baseline: 1.8063x; 1.8063x over previous
"""Trainium2 Bass kernel for nn_DiscriminativeLoss (v2).

Data-parallel over the batch axis: each of the 8 NeuronCores gets one sample.
Host ships two bf16 copies of the sample in window-major layout:

  xb [128, 16384]  : partition (32*jj+f), col u; n = 16384*jj + u   (bf16 x)
  xt0[128, 16384]  : same layout, x * t0 (masked copy; zero where t0=0)

Phase 1 (overlapped with the DMA, window = 2048 cols):
  s0pc[:, w] = reduce(xt0_w)          (DVE)     masked feature sums
  rs        = reduce(xb_w)            (DVE/ACT split)  total feature sums
  xsq_w     = xb_w^2 (bf16)           (ACT/Pool split)
Cross-partition finish via csel/cone PE matmuls -> m0, m1 on chip.

Phase 2: for each 512-col chunk i, two accumulating bf16 matmuls write
  PD_b[8*(i%16)+2*jj+c, q] = -2*m_c.x_n + q_n        (b = i//16, 2 banks)
ACT evacuates with func=Relu and per-partition bias ||m_c||^2 (safe max0),
then Sqrt / Relu(-dv) / Square on the packed [128, 1024] tile, and a DVE
tensor_tensor_reduce against a host-precomputed mask in the same packed
layout yields per-partition v contributions; a tiny PE matmul folds them
into v0, v1. Host combines the 8 per-core result vectors into the loss.
"""

import numpy as np
from contextlib import ExitStack

BS, NF, MAXC, NLOC = 8, 32, 4, 65536
DELTA_VAR, DELTA_DIST = 0.5, 1.5
ALPHA, BETA, GAMMA = 1.0, 1.0, 1e-4

NCORES = 8
U = NLOC // 4        # 16384 cols per core tile
WW = 2048            # DMA / phase-1 window
NW = U // WW         # 8 windows
CW = 512             # phase-2 chunk width
CH = U // CW         # 32 chunks

_CACHE = {}


def _host_constants():
    import ml_dtypes
    # csel: [128, 33]; col m<32 selects p%32==m, col 32 = ones
    sel33 = np.zeros((128, 33), dtype=np.float32)
    for p in range(128):
        sel33[p, p % 32] = 1.0
    sel33[:, 32] = 1.0
    ones33 = np.ones((128, 33), dtype=np.float32)
    # CB: [128, 2] col c = 1 where p%2 == c (cluster row selectors)
    cb = np.zeros((128, 2), dtype=np.float32)
    cb[0::2, 0] = 1.0
    cb[1::2, 1] = 1.0
    cst = np.concatenate([sel33, ones33, cb], axis=1)  # [128, 68]
    # ONESALL bf16 [128, 128]: slice s (cols 32s..32s+32) has quadrant
    # selectors in cols 8s+2jj+c only (zero elsewhere)
    ones8 = np.zeros((128, 8), dtype=np.float32)
    for jj in range(4):
        ones8[32 * jj:32 * jj + 32, 2 * jj] = 1.0
        ones8[32 * jj:32 * jj + 32, 2 * jj + 1] = 1.0
    onesall = np.zeros((128, 128), dtype=np.float32)
    for s in range(4):
        onesall[:, 32 * s + 8 * s:32 * s + 8 * s + 8] = ones8
    return {"cst": cst, "onesall": onesall.astype(ml_dtypes.bfloat16)}


def _emit(ctx, tc, xb_d, xt0_d, m_d, t0n_d, cst_d, onesall_d, res_d):
    import concourse.mybir as mybir

    nc = tc.nc
    f32 = mybir.dt.float32
    bf16 = mybir.dt.bfloat16
    Alu = mybir.AluOpType
    Act = mybir.ActivationFunctionType
    AxX = mybir.AxisListType.X

    persist = ctx.enter_context(tc.tile_pool(name="persist", bufs=1))
    scratch = ctx.enter_context(tc.tile_pool(name="scratch", bufs=1))
    p_dist = ctx.enter_context(tc.tile_pool(name="p_dist", bufs=2, space="PSUM"))
    p_fin = ctx.enter_context(tc.tile_pool(name="p_fin", bufs=2, space="PSUM"))

    def ptile(shape, tag, dtype=f32):
        return persist.tile(shape, dtype, tag=tag, name=tag)

    # ---- persistent tiles ----
    XB = ptile([128, U], "XB", dtype=bf16)
    XT0 = ptile([128, U], "XT0", dtype=bf16)
    XSQ = ptile([128, U], "XSQ", dtype=bf16)
    MSK = ptile([128, 3 * CW], "MSK")               # hinge mask, packed layout
    T0N = ptile([128, CW], "T0N")                   # t0 in [p, q] layout (fp32)
    SD = ptile([128, 3 * CW], "SD")                 # packed hinge values
    CST = ptile([128, 68], "CST")
    csel = CST[:, 0:33]
    cone = CST[:, 33:66]
    cb01 = CST[:, 66:68]
    ONESALL = ptile([128, 128], "ONESALL", dtype=bf16)
    W2B = ptile([128, 8], "W2B", dtype=bf16)
    W2ALL = ptile([128, 128], "W2ALL", dtype=bf16)
    MISC = ptile([128, 64], "MISC")
    s0pc = MISC[:, 0:NW]
    rspc = MISC[:, NW:3 * NW]                       # rs partials (DVE+ACT)
    stats3 = MISC[:, 24:27]
    stat_f = MISC[0:33, 27:30]
    stat_t = MISC[0:33, 30:33]
    cnts = MISC[0:32, 33:37]
    mraw = MISC[0:32, 37:39]
    mneg = MISC[0:32, 39:41]
    mnegb = persist.tile([32, 2], bf16, tag="mnegb", name="mnegb")
    csb = MISC[0:1, 41:43]
    biasv = MISC[:, 43:44]
    biasdv = MISC[:, 44:45]
    s1col = MISC[0:32, 45:46]
    vcol = MISC[:, 46:47]
    vout = MISC[0:2, 47:48]
    cb0m = MISC[:, 48:49]
    cb1m = MISC[:, 49:50]
    junk = scratch.tile([128, WW], bf16, tag="junk", name="junk")

    # ---- loads (all triggers on SP queue; windows first, mask/t0n last) ----
    nc.gpsimd.memset(biasdv, -DELTA_VAR)
    nc.gpsimd.memset(SD[:], 0.0)
    nc.sync.dma_start(CST[:], cst_d.ap())
    nc.sync.dma_start(ONESALL[:], onesall_d.ap())
    xb_ap = xb_d.ap()
    xt0_ap = xt0_d.ap()
    for w in range(NW):
        nc.sync.dma_start(XB[:, w * WW:(w + 1) * WW], xb_ap[w])
        nc.sync.dma_start(XT0[:, w * WW:(w + 1) * WW], xt0_ap[w])
    nc.sync.dma_start(T0N[:], t0n_d.ap())
    nc.sync.dma_start(MSK[:], m_d.ap())

    # ---- phase 1: per-window reduces + squares, overlapping the DMA ----
    # col split tuned to engine rates: DVE ~1.07ns/col, ACT ~0.83, Pool ~2.0
    RS_DVE = 512            # rs cols on DVE; rest (1536) on ACT copy+accum
    SQ_ACT = 1280           # square cols on ACT; rest (768) on Pool
    for w in range(NW):
        xbw = XB[:, w * WW:(w + 1) * WW]
        xtw = XT0[:, w * WW:(w + 1) * WW]
        xqw = XSQ[:, w * WW:(w + 1) * WW]
        # s0 partial: one DVE reduce over the masked copy
        nc.vector.reduce_sum(s0pc[:, w:w + 1], xtw, axis=AxX)
        # rs partials: DVE head + ACT tail (copy with accum, junk elementwise)
        nc.vector.reduce_sum(rspc[:, w:w + 1], xbw[:, 0:RS_DVE], axis=AxX)
        nc.scalar.activation(
            out=junk[:, 0:WW - RS_DVE], in_=xbw[:, RS_DVE:WW], func=Act.Copy,
            accum_out=rspc[:, NW + w:NW + w + 1])
        # squares: ACT head + Pool tail
        nc.scalar.activation(
            out=xqw[:, 0:SQ_ACT], in_=xbw[:, 0:SQ_ACT], func=Act.Square)
        nc.gpsimd.tensor_tensor(
            out=xqw[:, SQ_ACT:WW], in0=xbw[:, SQ_ACT:WW],
            in1=xbw[:, SQ_ACT:WW], op=Alu.mult)

    # ---- cross-partition finish: per-f sums + totals ----
    nc.vector.reduce_sum(stats3[:, 0:1], s0pc[:], axis=AxX)
    nc.vector.reduce_sum(stats3[:, 1:2], rspc[:], axis=AxX)
    nc.vector.reduce_sum(stats3[:, 2:3], T0N[:], axis=AxX)
    F1f = p_fin.tile([33, 3], f32, tag="fin")
    nc.tensor.matmul(F1f[:], csel, stats3, start=True, stop=True)
    F1t = p_fin.tile([33, 3], f32, tag="fin")
    nc.tensor.matmul(F1t[:], cone, stats3, start=True, stop=True)
    nc.scalar.copy(stat_f[:], F1f[:])
    nc.scalar.copy(stat_t[:], F1t[:])

    # ---- means ----
    cnt0col = stat_t[0:32, 2:3]
    nc.vector.tensor_scalar(
        out=cnts[:, 0:1], in0=cnt0col, scalar1=1.0, scalar2=None, op0=Alu.max)
    nc.vector.tensor_scalar(
        out=cnts[:, 1:2], in0=cnt0col, scalar1=-1.0, scalar2=float(NLOC),
        op0=Alu.mult, op1=Alu.add)
    nc.vector.tensor_scalar(
        out=cnts[:, 1:2], in0=cnts[:, 1:2], scalar1=1.0, scalar2=None, op0=Alu.max)
    nc.vector.reciprocal(cnts[:, 2:3], cnts[:, 0:1])
    nc.vector.reciprocal(cnts[:, 3:4], cnts[:, 1:2])
    nc.vector.tensor_tensor(
        out=mraw[:, 0:1], in0=stat_f[0:32, 0:1], in1=cnts[:, 2:3], op=Alu.mult)
    nc.vector.tensor_tensor(
        out=s1col, in0=stat_f[0:32, 1:2], in1=stat_f[0:32, 0:1], op=Alu.subtract)
    nc.vector.tensor_tensor(
        out=mraw[:, 1:2], in0=s1col, in1=cnts[:, 3:4], op=Alu.mult)
    nc.vector.tensor_scalar(
        out=mneg, in0=mraw, scalar1=-2.0, scalar2=None, op0=Alu.mult)

    # W2B: block-diagonal -2*m_c weights (bf16), cols 2*jj+c nonzero per block
    nc.vector.tensor_copy(mnegb[:], mneg)
    nc.gpsimd.memset(W2B[:], 0.0)
    dmaq = [nc.sync, nc.scalar, nc.sync, nc.scalar]
    for jj in range(4):
        dmaq[jj].dma_start(W2B[32 * jj:32 * jj + 32, 2 * jj:2 * jj + 2], mnegb[:])
    nc.gpsimd.memset(W2ALL[:], 0.0)
    for s in range(4):
        nc.vector.tensor_copy(
            W2ALL[:, 32 * s + 8 * s:32 * s + 8 * s + 8], W2B[:])

    # ---- ||m_c||^2 -> biasv (per-partition, alternating by p%2) ----
    mm0 = p_fin.tile([1, 2], f32, tag="fin")
    nc.tensor.matmul(mm0[:], mraw[:, 0:1], mraw, start=True, stop=True)
    mm1 = p_fin.tile([1, 2], f32, tag="fin")
    nc.tensor.matmul(mm1[:], mraw[:, 1:2], mraw, start=True, stop=True)
    nc.scalar.copy(csb[0:1, 0:1], mm0[0:1, 0:1])
    nc.scalar.copy(csb[0:1, 1:2], mm1[0:1, 1:2])
    nc.gpsimd.partition_broadcast(cb0m[:], csb[0:1, 0:1], channels=128)
    nc.gpsimd.partition_broadcast(cb1m[:], csb[0:1, 1:2], channels=128)
    nc.vector.tensor_tensor(out=cb0m, in0=cb0m, in1=cb01[:, 0:1], op=Alu.mult)
    nc.vector.tensor_tensor(out=cb1m, in0=cb1m, in1=cb01[:, 1:2], op=Alu.mult)
    nc.vector.tensor_tensor(out=biasv, in0=cb0m, in1=cb1m, op=Alu.add)

    res_ap = res_d.ap()
    nc.sync.dma_start(res_ap[0:64].rearrange("(c f) -> f c", c=2), mraw)

    # ---- phase 2: -2m.x + q into 3 packed PSUM tiles ----
    # chunk i = 12*T + 4*z + s -> tile T, out base 32*z, lhsT cols 8*s..
    # (4 chunks accumulate into each 32-row block; zero lhsT cols are inert)
    for T in range(3):
        nz = 3 if T < 2 else 2
        pd = p_dist.tile([128, CW], f32, tag="dist")
        for z in range(nz):
            for s in range(4):
                i = 12 * T + 4 * z + s
                nc.tensor.matmul(
                    pd[32 * z:32 * z + 32, :], W2ALL[:, 32 * s:32 * s + 32],
                    XB[:, i * CW:(i + 1) * CW], start=(s == 0), stop=False)
                nc.tensor.matmul(
                    pd[32 * z:32 * z + 32, :], ONESALL[:, 32 * s:32 * s + 32],
                    XSQ[:, i * CW:(i + 1) * CW], start=False, stop=(s == 3))
        # evacuate with +||m_c||^2 bias and max(.,0) in one ACT op
        nc.scalar.activation(
            out=SD[0:32 * nz, T * CW:(T + 1) * CW], in_=pd[0:32 * nz, :],
            func=Act.Relu, bias=biasv[0:32 * nz, 0:1])

    # ---- hinge chain on SD [128, 1024] ----
    nc.scalar.activation(out=SD, in_=SD, func=Act.Sqrt)
    nc.scalar.activation(out=SD, in_=SD, func=Act.Relu, bias=biasdv[:, 0:1])
    nc.scalar.activation(out=SD, in_=SD, func=Act.Square)
    sdm = scratch.tile([128, 3 * CW], f32, tag="sdm", name="sdm")
    nc.vector.tensor_tensor(out=sdm[:], in0=SD, in1=MSK[:], op=Alu.mult)
    nc.vector.reduce_sum(vcol, sdm[:], axis=AxX)

    # ---- v0/v1 + outputs ----
    F2 = p_fin.tile([2, 1], f32, tag="fin")
    nc.tensor.matmul(F2[:], cb01, vcol, start=True, stop=True)
    nc.scalar.copy(vout, F2[:])
    nc.sync.dma_start(res_ap[64:67], stat_t[0:1, 0:3])
    nc.sync.dma_start(res_ap[67:69].rearrange("(a b) -> a b", a=2), vout)


def _build():
    import concourse.bacc as bacc
    import concourse.tile as tile
    import concourse.mybir as mybir

    f32 = mybir.dt.float32
    bf16 = mybir.dt.bfloat16
    nc = bacc.Bacc("TRN2", target_bir_lowering=False, debug=False)
    xb_d = nc.dram_tensor("xb", [NW, 128, WW], bf16, kind="ExternalInput")
    xt0_d = nc.dram_tensor("xt0", [NW, 128, WW], bf16, kind="ExternalInput")
    m_d = nc.dram_tensor("msk", [128, 3 * CW], f32, kind="ExternalInput")
    t0n_d = nc.dram_tensor("t0n", [128, CW], f32, kind="ExternalInput")
    cst_d = nc.dram_tensor("cst", [128, 68], f32, kind="ExternalInput")
    onesall_d = nc.dram_tensor("onesall", [128, 128], bf16, kind="ExternalInput")
    res_d = nc.dram_tensor("res", [128], f32, kind="ExternalOutput")
    with tile.TileContext(nc) as tc:
        with ExitStack() as ctx:
            _emit(ctx, tc, xb_d, xt0_d, m_d, t0n_d, cst_d, onesall_d, res_d)
    nc.compile()
    return nc


def get_nc():
    if "nc" not in _CACHE:
        _CACHE["nc"] = _build()
    return _CACHE["nc"]


def make_in_maps(input, target):
    import ml_dtypes
    consts = _host_constants()
    in_maps = []
    p = np.arange(128)
    jj = (p >> 1) & 3
    c = p & 1
    r = p >> 3
    for bcore in range(input.shape[0]):
        x = np.asarray(input[bcore], dtype=np.float32)      # [32, 65536]
        t0 = np.asarray(target[bcore, 0], dtype=np.float32)  # [65536]
        # tile layout [128, 16384]: partition 32*jj+f, col u, n = 16384*jj+u
        xl = x.reshape(32, 4, U).transpose(1, 0, 2).reshape(128, U)
        t0l = t0.reshape(4, U)                               # [jj, u]
        xt0 = xl * t0l[:, None, :].repeat(32, 1).reshape(128, U)
        # window-major DRAM: [NW, 128, WW]
        xb_w = xl.reshape(128, NW, WW).transpose(1, 0, 2)
        xt0_w = xt0.reshape(128, NW, WW).transpose(1, 0, 2)
        # hinge mask [128, 1536]: col 512*T+q ; i = 12*T + 4*z + s
        # p = 32*z + 8*s + 2*jj + c ; n = 16384*jj + 512*i + q ; value t_c(n)
        msk = np.zeros((128, 3 * CW), dtype=np.float32)
        q = np.arange(CW)
        z = p >> 5
        s = (p >> 3) & 3
        for T in range(3):
            nz = 3 if T < 2 else 2
            rows = p[p < 32 * nz]
            i = 12 * T + 4 * z[rows] + s[rows]
            n = 16384 * jj[rows, None] + 512 * i[:, None] + q[None, :]
            t = t0[n]
            msk[rows, T * CW:(T + 1) * CW] = np.where(
                c[rows, None] == 0, t, 1.0 - t)
        # t0 in [p, q] layout: t0n[p, q] = t0[512*p + q]
        t0n = t0.reshape(128, CW)
        m = {
            "xb": np.ascontiguousarray(xb_w).astype(ml_dtypes.bfloat16),
            "xt0": np.ascontiguousarray(xt0_w).astype(ml_dtypes.bfloat16),
            "msk": msk,
            "t0n": np.ascontiguousarray(t0n),
        }
        m.update(consts)
        in_maps.append(m)
    return in_maps


def combine_host(results, n_clusters):
    """results: list of 8 dicts with 'res' vectors. Returns scalar loss."""
    total = 0.0
    for b in range(BS):
        res = np.asarray(results[b]["res"], dtype=np.float64)
        m0, m1 = res[0:32], res[32:64]
        cnt0 = res[66]
        v0, v1 = res[67], res[68]
        ncb = float(n_clusters[b])
        counts = np.array([cnt0, NLOC - cnt0])
        active = counts > 0
        safe = np.where(active, counts, 1.0)
        c_var = float(np.where(active, np.array([v0, v1]) / safe, 0.0).sum())
        l_var = c_var / ncb
        dn = float(np.sqrt(((m0 - m1) ** 2).sum()))
        c_dist = 2.0 * max(2.0 * DELTA_DIST - dn, 0.0) ** 2
        l_dist = c_dist / (2.0 * ncb * (ncb - 1.0))
        l_reg = 0.5 * (np.sqrt((m0 ** 2).sum()) + np.sqrt((m1 ** 2).sum()))
        total += ALPHA * l_var + BETA * l_dist + GAMMA * l_reg
    return np.float32(total / BS)


def kernel(input, target, n_clusters):
    from concourse import bass_utils

    nc = get_nc()
    in_maps = make_in_maps(np.asarray(input), np.asarray(target))
    br = bass_utils.run_bass_kernel_spmd(nc, in_maps, core_ids=list(range(NCORES)))
    loss = combine_host(br.results, np.asarray(n_clusters))
    return np.array(loss, dtype=np.float32)


# revision 22
# speedup vs baseline: 2.4475x; 1.3550x over previous
"""Trainium2 Bass kernel for nn_DiscriminativeLoss (v2).

Data-parallel over the batch axis: each of the 8 NeuronCores gets one sample.
Host ships two bf16 copies of the sample in window-major layout:

  xb [128, 16384]  : partition (32*jj+f), col u; n = 16384*jj + u   (bf16 x)
  xt0[128, 16384]  : same layout, x * t0 (masked copy; zero where t0=0)

Phase 1 (overlapped with the DMA, window = 2048 cols):
  s0pc[:, w] = reduce(xt0_w)          (DVE)     masked feature sums
  rs        = reduce(xb_w)            (DVE/ACT split)  total feature sums
  xsq_w     = xb_w^2 (bf16)           (ACT/Pool split)
Cross-partition finish via csel/cone PE matmuls -> m0, m1 on chip.

Phase 2: for each 512-col chunk i, two accumulating bf16 matmuls write
  PD_b[8*(i%16)+2*jj+c, q] = -2*m_c.x_n + q_n        (b = i//16, 2 banks)
ACT evacuates with func=Relu and per-partition bias ||m_c||^2 (safe max0),
then Sqrt / Relu(-dv) / Square on the packed [128, 1024] tile, and a DVE
tensor_tensor_reduce against a host-precomputed mask in the same packed
layout yields per-partition v contributions; a tiny PE matmul folds them
into v0, v1. Host combines the 8 per-core result vectors into the loss.
"""

import numpy as np
from contextlib import ExitStack

BS, NF, MAXC, NLOC = 8, 32, 4, 65536
DELTA_VAR, DELTA_DIST = 0.5, 1.5
ALPHA, BETA, GAMMA = 1.0, 1.0, 1e-4

NCORES = 8
U = NLOC // 4        # 16384 cols per core tile
WW = 2048            # DMA / phase-1 window
NW = U // WW         # 8 windows
CW = 512             # phase-2 chunk width
CH = U // CW         # 32 chunks

_CACHE = {}


def _host_constants():
    import ml_dtypes
    # csel: [128, 33]; col m<32 selects p%32==m, col 32 = ones
    sel33 = np.zeros((128, 33), dtype=np.float32)
    for p in range(128):
        sel33[p, p % 32] = 1.0
    sel33[:, 32] = 1.0
    ones33 = np.ones((128, 33), dtype=np.float32)
    # CB: [128, 2] col c = 1 where p%2 == c (cluster row selectors)
    cb = np.zeros((128, 2), dtype=np.float32)
    cb[0::2, 0] = 1.0
    cb[1::2, 1] = 1.0
    cst = np.concatenate([sel33, ones33, cb], axis=1)  # [128, 68]
    # ONESALL bf16 [128, 128]: slice s (cols 32s..32s+32) has quadrant
    # selectors in cols 8s+2jj+c only (zero elsewhere)
    ones8 = np.zeros((128, 8), dtype=np.float32)
    for jj in range(4):
        ones8[32 * jj:32 * jj + 32, 2 * jj] = 1.0
        ones8[32 * jj:32 * jj + 32, 2 * jj + 1] = 1.0
    onesall = np.zeros((128, 128), dtype=np.float32)
    for s in range(4):
        onesall[:, 32 * s + 8 * s:32 * s + 8 * s + 8] = ones8
    return {"cst": cst, "onesall": onesall.astype(ml_dtypes.bfloat16),
            "eye32": np.eye(32, dtype=np.float32).astype(ml_dtypes.bfloat16)}


def _emit(ctx, tc, xb_d, xt0_d, m_d, t0n_d, cst_d, onesall_d, eye32_d, res_d):
    import concourse.mybir as mybir

    nc = tc.nc
    f32 = mybir.dt.float32
    bf16 = mybir.dt.bfloat16
    Alu = mybir.AluOpType
    Act = mybir.ActivationFunctionType
    AxX = mybir.AxisListType.X

    persist = ctx.enter_context(tc.tile_pool(name="persist", bufs=1))
    scratch = ctx.enter_context(tc.tile_pool(name="scratch", bufs=1))
    p_dist = ctx.enter_context(tc.tile_pool(name="p_dist", bufs=1, space="PSUM"))
    p_fin = ctx.enter_context(tc.tile_pool(name="p_fin", bufs=1, space="PSUM"))

    def ptile(shape, tag, dtype=f32):
        return persist.tile(shape, dtype, tag=tag, name=tag)

    # ---- persistent tiles ----
    XB = ptile([128, U], "XB", dtype=bf16)
    XT0 = ptile([128, U], "XT0", dtype=bf16)
    XSQ = ptile([128, U], "XSQ", dtype=bf16)
    MSK = ptile([128, 3 * CW], "MSK")               # hinge mask, packed layout
    T0N = ptile([128, CW], "T0N")                   # t0 in [p, q] layout (fp32)
    SD = ptile([128, 3 * CW], "SD")                 # packed hinge values
    SDQ = ptile([128, 3 * CW], "SDQ")               # packed q_n from P1
    CST = ptile([128, 68], "CST")
    csel = CST[:, 0:33]
    cone = CST[:, 33:66]
    cb01 = CST[:, 66:68]
    ONESALL = ptile([128, 128], "ONESALL", dtype=bf16)
    EYE32 = ptile([32, 32], "EYE32", dtype=bf16)
    W2B = ptile([128, 8], "W2B", dtype=bf16)
    W2ALL = ptile([128, 128], "W2ALL", dtype=bf16)
    MISC = ptile([128, 64], "MISC")
    s0pc = MISC[:, 0:NW]
    rspc = MISC[:, NW:3 * NW]                       # rs partials (DVE+ACT)
    stats3 = MISC[:, 24:27]
    stat_f = MISC[0:33, 27:30]
    stat_t = MISC[0:33, 30:33]
    cnts = MISC[0:32, 33:37]
    mraw = MISC[0:32, 37:39]
    mnegb = persist.tile([32, 2], bf16, tag="mnegb", name="mnegb")
    csb = MISC[0:1, 41:43]
    biasv = MISC[:, 43:44]
    biasdv = MISC[:, 44:45]
    s1col = MISC[0:32, 45:46]
    vcolT = MISC[:, 54:57]
    vout = MISC[0:2, 57:60]
    cb0m = MISC[:, 48:49]
    cb1m = MISC[:, 49:50]
    junk = scratch.tile([128, WW], bf16, tag="junk", name="junk")

    # ---- loads (all triggers on SP queue; windows first, mask/t0n last) ----
    nc.gpsimd.memset(biasdv, -DELTA_VAR)
    nc.gpsimd.memset(SD[:], 0.0)
    nc.gpsimd.memset(MISC[:, 60:64], 1.0)
    nc.scalar.activation(out=MISC[0:1, 62:63], in_=MISC[0:1, 63:64],
                         func=Act.Sqrt)
    nc.gpsimd.memset(W2B[:], 0.0)
    nc.gpsimd.memset(W2ALL[:], 0.0)
    nc.gpsimd.memset(MISC[:, 54:57], 0.0)
    nc.sync.dma_start(CST[:], cst_d.ap())
    nc.sync.dma_start(ONESALL[:], onesall_d.ap())
    nc.sync.dma_start(EYE32[:], eye32_d.ap())
    xb_ap = xb_d.ap()
    xt0_ap = xt0_d.ap()
    for w in range(NW):
        nc.sync.dma_start(XB[:, w * WW:(w + 1) * WW], xb_ap[w])
        nc.sync.dma_start(XT0[:, w * WW:(w + 1) * WW], xt0_ap[w])
    nc.sync.dma_start(T0N[:], t0n_d.ap())
    nc.sync.dma_start(MSK[:], m_d.ap())

    pdt = [p_dist.tile([128, CW], f32, tag=f"dist{t}", name=f"pd{t}")
           for t in range(3)]

    # ---- phase 1: per-window reduces + squares, overlapping the DMA ----
    # col split tuned to engine rates: DVE ~1.07ns/col, ACT ~0.83, Pool ~2.0
    RS_DVE = 512            # rs cols on DVE; rest (1536) on ACT copy+accum
    SQ_ACT = 1280           # square cols on ACT; rest (768) on Pool
    for w in range(NW):
        xbw = XB[:, w * WW:(w + 1) * WW]
        xtw = XT0[:, w * WW:(w + 1) * WW]
        xqw = XSQ[:, w * WW:(w + 1) * WW]
        # s0 partial: one DVE reduce over the masked copy
        nc.vector.reduce_sum(s0pc[:, w:w + 1], xtw, axis=AxX)
        # rs partials: DVE head + ACT tail (copy with accum, junk elementwise)
        nc.vector.reduce_sum(rspc[:, w:w + 1], xbw[:, 0:RS_DVE], axis=AxX)
        nc.scalar.activation(
            out=junk[:, 0:WW - RS_DVE], in_=xbw[:, RS_DVE:WW], func=Act.Copy,
            accum_out=rspc[:, NW + w:NW + w + 1])
        # squares: ACT head + Pool tail
        nc.scalar.activation(
            out=xqw[:, 0:SQ_ACT], in_=xbw[:, 0:SQ_ACT], func=Act.Square)
        nc.gpsimd.tensor_tensor(
            out=xqw[:, SQ_ACT:WW], in0=xbw[:, SQ_ACT:WW],
            in1=xbw[:, SQ_ACT:WW], op=Alu.mult)
        # q_n accumulation: window w = block (T=w//3, z=w%3); 4 ones-matmuls
        # (no means needed) start the PSUM tiles and keep PE warm
        T, z = w // 3, w % 3
        pd = pdt[T]
        for s in range(4):
            i = 12 * T + 4 * z + s
            nc.tensor.matmul(
                pd[32 * z:32 * z + 32, :], ONESALL[:, 32 * s:32 * s + 32],
                XSQ[:, i * CW:(i + 1) * CW], start=(s == 0), stop=(s == 3))
        nz = 3 if T < 2 else 2
        if z == nz - 1:
            nc.scalar.copy(SDQ[0:32 * nz, T * CW:(T + 1) * CW],
                           pd[0:32 * nz, :])

    # ---- cross-partition finish: per-f sums + totals ----
    nc.vector.reduce_sum(stats3[:, 0:1], s0pc[:], axis=AxX)
    nc.vector.reduce_sum(stats3[:, 1:2], rspc[:], axis=AxX)
    nc.vector.reduce_sum(stats3[:, 2:3], T0N[:], axis=AxX)
    F1f = p_fin.tile([33, 3], f32, tag="fin")
    nc.tensor.matmul(F1f[:], csel, stats3, start=True, stop=True)
    F1t = p_fin.tile([33, 3], f32, tag="fin")
    nc.tensor.matmul(F1t[:], cone, stats3, start=True, stop=True)
    nc.scalar.copy(stat_f[:], F1f[:])
    nc.scalar.copy(stat_t[:], F1t[:])

    # ---- means ----
    cnt0col = stat_t[0:32, 2:3]
    nc.vector.tensor_scalar(
        out=cnts[:, 0:1], in0=cnt0col, scalar1=1.0, scalar2=None, op0=Alu.max)
    nc.vector.tensor_scalar(
        out=cnts[:, 1:2], in0=cnt0col, scalar1=-1.0, scalar2=float(NLOC),
        op0=Alu.mult, op1=Alu.add)
    nc.vector.tensor_scalar(
        out=cnts[:, 1:2], in0=cnts[:, 1:2], scalar1=1.0, scalar2=None, op0=Alu.max)
    nc.vector.reciprocal(cnts[:, 2:3], cnts[:, 0:1])
    nc.vector.reciprocal(cnts[:, 3:4], cnts[:, 1:2])
    nc.vector.tensor_tensor(
        out=mraw[:, 0:1], in0=stat_f[0:32, 0:1], in1=cnts[:, 2:3], op=Alu.mult)
    nc.vector.tensor_tensor(
        out=s1col, in0=stat_f[0:32, 1:2], in1=stat_f[0:32, 0:1], op=Alu.subtract)
    nc.vector.tensor_tensor(
        out=mraw[:, 1:2], in0=s1col, in1=cnts[:, 3:4], op=Alu.mult)
    nc.vector.tensor_scalar(
        out=mnegb[:], in0=mraw, scalar1=-2.0, scalar2=None, op0=Alu.mult)

    # W2 block-diagonal -2*m_c weights via PE: out[32jj+f, 2jj+c] = mnegb[f,c]
    # (two [64, 8] PSUM tiles; matmul out base must be 0 or 32 within each;
    # only the written 2-col blocks are evacuated -- W2B was zeroed early)
    w2p = [p_fin.tile([64, 8], f32, tag=f"w2p{h}", name=f"w2p{h}")
           for h in range(2)]
    for jj in range(4):
        h, zz = jj // 2, (jj % 2) * 32
        nc.tensor.matmul(
            w2p[h][zz:zz + 32, 2 * jj:2 * jj + 2], EYE32[:], mnegb[:],
            start=True, stop=True)
    for jj in range(4):
        h, zz = jj // 2, (jj % 2) * 32
        nc.scalar.copy(
            W2B[32 * jj:32 * jj + 32, 2 * jj:2 * jj + 2],
            w2p[h][zz:zz + 32, 2 * jj:2 * jj + 2])
    for s in range(4):
        nc.vector.tensor_copy(
            W2ALL[:, 32 * s + 8 * s:32 * s + 8 * s + 8], W2B[:])

    # ---- ||m_c||^2 -> biasv (per-partition, alternating by p%2) ----
    mm0 = p_fin.tile([1, 2], f32, tag="fin")
    nc.tensor.matmul(mm0[:], mraw[:, 0:1], mraw, start=True, stop=True)
    mm1 = p_fin.tile([1, 2], f32, tag="fin")
    nc.tensor.matmul(mm1[:], mraw[:, 1:2], mraw, start=True, stop=True)
    nc.scalar.copy(csb[0:1, 0:1], mm0[0:1, 0:1])
    nc.scalar.copy(csb[0:1, 1:2], mm1[0:1, 1:2])
    nc.gpsimd.partition_broadcast(cb0m[:], csb[0:1, 0:1], channels=128)
    nc.gpsimd.partition_broadcast(cb1m[:], csb[0:1, 1:2], channels=128)
    nc.vector.tensor_tensor(out=cb0m, in0=cb0m, in1=cb01[:, 0:1], op=Alu.mult)
    nc.vector.tensor_tensor(out=cb1m, in0=cb1m, in1=cb01[:, 1:2], op=Alu.mult)
    nc.vector.tensor_tensor(out=biasv, in0=cb0m, in1=cb1m, op=Alu.add)

    res_ap = res_d.ap()
    nc.sync.dma_start(res_ap[0:64].rearrange("(c f) -> f c", c=2), mraw)

    # ---- phase 2: remaining -2m.x matmuls, then per-T hinge pipeline ----
    for T in range(3):
        nz = 3 if T < 2 else 2
        pd = pdt[T]
        for z in range(nz):
            for s in range(4):
                i = 12 * T + 4 * z + s
                nc.tensor.matmul(
                    pd[32 * z:32 * z + 32, :], W2ALL[:, 32 * s:32 * s + 32],
                    XB[:, i * CW:(i + 1) * CW], start=(s == 0), stop=(s == 3))
        # d^2 = q + (-2m.x); +||m_c||^2 bias and max(.,0); hinge chain +
        # masked reduce on this T's 512 columns while PE works on T+1
        sdT = SD[0:32 * nz, T * CW:(T + 1) * CW]
        nc.vector.tensor_tensor(
            out=sdT, in0=SDQ[0:32 * nz, T * CW:(T + 1) * CW],
            in1=pd[0:32 * nz, :], op=Alu.add)
        nc.scalar.activation(
            out=sdT, in_=sdT, func=Act.Relu, bias=biasv[0:32 * nz, 0:1])
        nc.scalar.activation(out=sdT, in_=sdT, func=Act.Sqrt)
        nc.scalar.activation(out=sdT, in_=sdT, func=Act.Relu,
                             bias=biasdv[0:32 * nz, 0:1])
        nc.scalar.activation(out=sdT, in_=sdT, func=Act.Square)
        sdm = scratch.tile([128, CW], f32, tag="sdm", name="sdm")
        nc.vector.tensor_tensor(
            out=sdm[0:32 * nz, :], in0=sdT,
            in1=MSK[0:32 * nz, T * CW:(T + 1) * CW], op=Alu.mult)
        nc.vector.reduce_sum(vcolT[0:32 * nz, T:T + 1], sdm[0:32 * nz, :],
                             axis=AxX)

    # ---- v0/v1 + outputs ----
    F2 = p_fin.tile([2, 3], f32, tag="fin")
    nc.tensor.matmul(F2[:], cb01, vcolT[:], start=True, stop=True)
    nc.scalar.copy(vout, F2[:])
    nc.sync.dma_start(res_ap[64:67], stat_t[0:1, 0:3])
    nc.sync.dma_start(res_ap[67:73].rearrange("(a b) -> a b", a=2), vout)


def _build():
    import concourse.bacc as bacc
    import concourse.tile as tile
    import concourse.mybir as mybir

    f32 = mybir.dt.float32
    bf16 = mybir.dt.bfloat16
    nc = bacc.Bacc("TRN2", target_bir_lowering=False, debug=False)
    xb_d = nc.dram_tensor("xb", [NW, 128, WW], bf16, kind="ExternalInput")
    xt0_d = nc.dram_tensor("xt0", [NW, 128, WW], bf16, kind="ExternalInput")
    m_d = nc.dram_tensor("msk", [128, 3 * CW], f32, kind="ExternalInput")
    t0n_d = nc.dram_tensor("t0n", [128, CW], f32, kind="ExternalInput")
    cst_d = nc.dram_tensor("cst", [128, 68], f32, kind="ExternalInput")
    onesall_d = nc.dram_tensor("onesall", [128, 128], bf16, kind="ExternalInput")
    eye32_d = nc.dram_tensor("eye32", [32, 32], bf16, kind="ExternalInput")
    res_d = nc.dram_tensor("res", [128], f32, kind="ExternalOutput")
    with tile.TileContext(nc) as tc:
        with ExitStack() as ctx:
            _emit(ctx, tc, xb_d, xt0_d, m_d, t0n_d, cst_d, onesall_d, eye32_d,
                  res_d)
    nc.compile()
    return nc


def get_nc():
    if "nc" not in _CACHE:
        _CACHE["nc"] = _build()
    return _CACHE["nc"]


def make_in_maps(input, target):
    import ml_dtypes
    consts = _host_constants()
    in_maps = []
    p = np.arange(128)
    jj = (p >> 1) & 3
    c = p & 1
    for bcore in range(input.shape[0]):
        x = np.asarray(input[bcore], dtype=np.float32)      # [32, 65536]
        t0 = np.asarray(target[bcore, 0], dtype=np.float32)  # [65536]
        # tile layout [128, 16384]: partition 32*jj+f, col u, n = 16384*jj+u
        xl = x.reshape(32, 4, U).transpose(1, 0, 2).reshape(128, U)
        t0l = t0.reshape(4, U)                               # [jj, u]
        xt0 = xl * t0l[:, None, :].repeat(32, 1).reshape(128, U)
        # window-major DRAM: [NW, 128, WW]
        xb_w = xl.reshape(128, NW, WW).transpose(1, 0, 2)
        xt0_w = xt0.reshape(128, NW, WW).transpose(1, 0, 2)
        # hinge mask [128, 1536]: col 512*T+q ; i = 12*T + 4*z + s
        # p = 32*z + 8*s + 2*jj + c ; n = 16384*jj + 512*i + q ; value t_c(n)
        msk = np.zeros((128, 3 * CW), dtype=np.float32)
        q = np.arange(CW)
        z = p >> 5
        s = (p >> 3) & 3
        for T in range(3):
            nz = 3 if T < 2 else 2
            rows = p[p < 32 * nz]
            i = 12 * T + 4 * z[rows] + s[rows]
            n = 16384 * jj[rows, None] + 512 * i[:, None] + q[None, :]
            t = t0[n]
            msk[rows, T * CW:(T + 1) * CW] = np.where(
                c[rows, None] == 0, t, 1.0 - t)
        t0n = t0.reshape(128, CW)
        m = {
            "xb": np.ascontiguousarray(xb_w).astype(ml_dtypes.bfloat16),
            "xt0": np.ascontiguousarray(xt0_w).astype(ml_dtypes.bfloat16),
            "msk": msk,
            "t0n": np.ascontiguousarray(t0n),
        }
        m.update(consts)
        in_maps.append(m)
    return in_maps


def combine_host(results, n_clusters):
    """results: list of 8 dicts with 'res' vectors. Returns scalar loss."""
    total = 0.0
    for b in range(BS):
        res = np.asarray(results[b]["res"], dtype=np.float64)
        m0, m1 = res[0:32], res[32:64]
        cnt0 = res[66]
        v0, v1 = res[67:70].sum(), res[70:73].sum()
        ncb = float(n_clusters[b])
        counts = np.array([cnt0, NLOC - cnt0])
        active = counts > 0
        safe = np.where(active, counts, 1.0)
        c_var = float(np.where(active, np.array([v0, v1]) / safe, 0.0).sum())
        l_var = c_var / ncb
        dn = float(np.sqrt(((m0 - m1) ** 2).sum()))
        c_dist = 2.0 * max(2.0 * DELTA_DIST - dn, 0.0) ** 2
        l_dist = c_dist / (2.0 * ncb * (ncb - 1.0))
        l_reg = 0.5 * (np.sqrt((m0 ** 2).sum()) + np.sqrt((m1 ** 2).sum()))
        total += ALPHA * l_var + BETA * l_dist + GAMMA * l_reg
    return np.float32(total / BS)


def kernel(input, target, n_clusters):
    from concourse import bass_utils

    nc = get_nc()
    in_maps = make_in_maps(np.asarray(input), np.asarray(target))
    br = bass_utils.run_bass_kernel_spmd(nc, in_maps, core_ids=list(range(NCORES)))
    loss = combine_host(br.results, np.asarray(n_clusters))
    return np.array(loss, dtype=np.float32)
